# revision 1
# baseline (speedup 1.0000x reference)
"""Trainium2 (8 NeuronCores) kernel for nn_BlockModel_9758165696627.

GNN message passing: 2 residual blocks of
  gather(nbr) + gather(self) + add_info -> MLP(relu) -> segment_max -> @Wo + residual
then a final 129 -> 64 -> 1 MLP.

Strategy (node/segment sharding):
  N = 15872 nodes, E = 253952 edges. Each of the 8 cores owns N/8 = 1984
  contiguous segments; since self_indexes are sorted, each core owns a
  contiguous slice of edges and the segment max is entirely local.

  First-layer refactor: x @ W1 = A[nbr] + B[self] + w1r*add_info where
  A = interp @ W1[:129], B = interp @ W1[129:258] + b1, w1r = W1[258].
  Each core computes A rows for its own nodes (small matmuls), the rows are
  AllGathered into a bf16 table, and per-edge rows are fetched with
  gpsimd.dma_gather (transpose mode: feature-on-partition, edge-on-free).
  All gathers run on SWDGE queue 0: concurrent transpose gathers on
  multiple queues interleave their xbar sprays and corrupt data.

  B[self] is segment-constant and self-indices are local, so B' is kept
  on-chip feature-major and broadcast into the edge tiles with 0-stride
  access patterns -- no second gather.

  Edges are laid out region-major (grouped by segment size, no padding),
  which turns the ragged segment max into a few strided tensor_reduce
  calls and makes the B broadcast a strided AP.
"""

import numpy as np
import ml_dtypes

BF16 = ml_dtypes.bfloat16

N = 15872
D = 129
H = 128
NCORES = 8
NPC = N // NCORES  # 1984 nodes per core
TILE_SLOTS = 4352  # max edge slots per gather tile


# ---------------------------------------------------------------------------
# host-side preprocessing
# ---------------------------------------------------------------------------

def _arith_runs(nodes):
    """Split a sorted node list into (start, stride, count) arithmetic runs."""
    runs = []
    i = 0
    n = len(nodes)
    while i < n:
        if i + 1 == n:
            runs.append((int(nodes[i]), 1, 1))
            break
        stride = int(nodes[i + 1] - nodes[i])
        j = i + 1
        while j + 1 < n and nodes[j + 1] - nodes[j] == stride:
            j += 1
        runs.append((int(nodes[i]), stride, j - i + 1))
        i = j + 1
    return runs


def _layout(sizes_core):
    """Region-major tile/piece structure for one core's segment sizes.

    Returns list of tiles; each tile is dict(width, real, pieces) where width
    is the padded gather size (%128) and pieces are
    (size, nseg, node0, node_stride, slot_off) with slots of each piece
    contiguous: segments of equal size, node ids in an arithmetic run.
    """
    order = []  # (size, node0, stride, nseg) pieces over the whole core
    for s in np.unique(sizes_core):
        nodes = np.where(sizes_core == s)[0]
        for (n0, st, cnt) in _arith_runs(nodes):
            order.append((int(s), n0, st, cnt))

    tiles = []
    cur = []
    cur_slots = 0

    def flush():
        nonlocal cur, cur_slots
        if cur:
            w = -(-cur_slots // 128) * 128
            tiles.append(dict(width=w, real=cur_slots, pieces=cur))
            cur, cur_slots = [], 0

    for (s, n0, st, cnt) in order:
        done = 0
        while done < cnt:
            room = (TILE_SLOTS - cur_slots) // s
            if room == 0:
                flush()
                room = TILE_SLOTS // s
            take = min(cnt - done, room)
            cur.append((s, take, n0 + done * st, st, cur_slots))
            cur_slots += take * s
            done += take
    flush()
    # smallest tile first: its gather desc-gen is short, so the compute
    # pipeline starts sooner after each AllGather
    tiles.sort(key=lambda t: t["width"])
    # split the final tile near its midpoint (at a segment boundary) so the
    # end-of-block drain chain runs on a half-size tile
    last = tiles.pop()
    half, apieces, bpieces = 0, [], []
    for (s, nseg, node0, nstride, sloff) in last["pieces"]:
        if half + nseg * s <= last["real"] // 2 + s:
            apieces.append((s, nseg, node0, nstride, half))
            half += nseg * s
        else:
            take = max(0, (last["real"] // 2 - half) // s)
            if take:
                apieces.append((s, take, node0, nstride, half))
                half += take * s
            boff = sum(p[0] * p[1] for p in bpieces)
            bpieces.append((s, nseg - take, node0 + take * nstride, nstride,
                            boff))
    def halve(tile_pieces):
        half, ap_, bp_ = 0, [], []
        total = sum(p[0] * p[1] for p in tile_pieces)
        for (s, nseg, node0, nstride, sloff) in tile_pieces:
            if half + nseg * s <= total // 2 + s:
                ap_.append((s, nseg, node0, nstride, half))
                half += nseg * s
            else:
                take = max(0, (total // 2 - half) // s)
                if take:
                    ap_.append((s, take, node0, nstride, half))
                    half += take * s
                boff = sum(p[0] * p[1] for p in bp_)
                bp_.append((s, nseg - take, node0 + take * nstride, nstride,
                            boff))
        return ap_, bp_

    for pieces in (apieces, *halve(bpieces)):
        realn = sum(p[0] * p[1] for p in pieces)
        if realn:
            tiles.append(dict(width=-(-realn // 128) * 128, real=realn,
                              pieces=pieces))
    return tiles


def _preprocess(sizes, nbr, a):
    csum = np.zeros(N + 1, np.int64)
    np.cumsum(sizes, out=csum[1:])

    sizes0 = sizes[:NPC]
    uniform = all(
        np.array_equal(sizes[c * NPC:(c + 1) * NPC], sizes0)
        for c in range(NCORES)
    )
    assert uniform, "per-core segment-size patterns differ; unsupported"
    tiles = _layout(sizes0)
    SP = sum(t["width"] for t in tiles)

    def wrap16(idx):
        n = len(idx)
        assert n % 16 == 0
        w = idx.astype(np.int16).reshape(n // 16, 16).T
        return np.ascontiguousarray(np.tile(w, (8, 1)))

    cores = []
    for c in range(NCORES):
        idx_stream = np.empty(SP, np.int64)
        a_stream = np.zeros(SP, np.float32)
        off = 0
        for t in tiles:
            for (s, nseg, node0, nstride, slot_off) in t["pieces"]:
                for k in range(nseg):
                    g = c * NPC + node0 + k * nstride
                    e0, e1 = csum[g], csum[g + 1]
                    assert e1 - e0 == s
                    p = off + slot_off + k * s
                    idx_stream[p:p + s] = nbr[e0:e1]
                    a_stream[p:p + s] = a[e0:e1]
            # pad tail with a repeat of the tile's first index (harmless)
            pr = off + t["real"]
            idx_stream[pr:off + t["width"]] = idx_stream[off]
            off += t["width"]
        assert off == SP
        cores.append(dict(idxA=wrap16(idx_stream),
                          a_pad=a_stream.astype(BF16)[None, :]))

    struct = dict(SP=SP, tiles=tiles)
    return cores, struct


def _weights_inputs(inputs):
    out = {}
    for b in (0, 1):
        W1 = np.asarray(inputs[f"b{b}_W1"], np.float32)
        b1 = np.asarray(inputs[f"b{b}_b1"], np.float32)
        W2 = np.asarray(inputs[f"b{b}_W2"], np.float32)
        b2 = np.asarray(inputs[f"b{b}_b2"], np.float32)
        Wo = np.asarray(inputs[f"b{b}_Wo"], np.float32)
        bo = np.asarray(inputs[f"b{b}_bo"], np.float32)
        p = f"k{b}_"
        out[p + "w1top_m"] = W1[0:128].astype(BF16)
        out[p + "w1top_l"] = W1[128:129].astype(BF16)
        out[p + "w1mid_m"] = W1[D:D + 128].astype(BF16)
        out[p + "w1mid_l"] = W1[D + 128:D + 129].astype(BF16)
        out[p + "b1col"] = b1[:, None].copy()
        out[p + "w1r"] = W1[2 * D:2 * D + 1].astype(BF16)
        out[p + "w2"] = W2.astype(BF16)
        out[p + "b2col"] = b2[:, None].copy()
        out[p + "wo_m"] = Wo[:, 0:128].astype(BF16)
        out[p + "wo_l"] = Wo[:, 128:129].astype(BF16)
        out[p + "bo_m"] = bo[0:128, None].copy()
        out[p + "bo_l"] = bo[128:129, None].copy()
    out["fw1_m"] = np.asarray(inputs["f_W1"], np.float32)[0:128].copy()
    out["fw1_l"] = np.asarray(inputs["f_W1"], np.float32)[128:129].copy()
    out["fb1col"] = np.asarray(inputs["f_b1"], np.float32)[:, None].copy()
    out["fw2"] = np.asarray(inputs["f_W2"], np.float32).copy()
    out["fb2"] = np.asarray(inputs["f_b2"], np.float32)[:, None].copy()
    return out


# ---------------------------------------------------------------------------
# bass graph
# ---------------------------------------------------------------------------

def build_graph(struct):
    import concourse.bacc as bacc
    import concourse.bass as bass
    import concourse.mybir as mybir
    import concourse.tile as tile
    from contextlib import ExitStack

    f32 = mybir.dt.float32
    bf16 = mybir.dt.bfloat16
    i16 = mybir.dt.int16
    Alu = mybir.AluOpType
    Act = mybir.ActivationFunctionType

    SP = struct["SP"]
    tiles = struct["tiles"]
    NPCP = NPC + 64  # padded bT width for strided-view headroom

    nc = bacc.Bacc("TRN2", target_bir_lowering=False, debug=False,
                   num_devices=NCORES)

    din = {}
    def dparam(name, shape, dtype):
        din[name] = nc.dram_tensor(name, list(shape), dtype, kind="ExternalInput")
        return din[name]

    dparam("idxA", (128, SP // 16), i16)
    dparam("a_pad", (1, SP), bf16)
    dparam("interpT", (D, NPC), f32)
    dparam("interpTb", (D, NPC), bf16)
    wnames_bf = ["w1top_m", "w1top_l", "w1mid_m", "w1mid_l", "w1r",
                 "w2", "wo_m", "wo_l"]
    wnames_f32 = ["b1col", "b2col", "bo_m", "bo_l"]
    shapes = dict(w1top_m=(128, 128), w1top_l=(1, 128), w1mid_m=(128, 128),
                  w1mid_l=(1, 128), w1r=(1, 128), w2=(128, 128),
                  wo_m=(128, 128), wo_l=(128, 1), b1col=(128, 1),
                  b2col=(128, 1), bo_m=(128, 1), bo_l=(1, 1))
    for b in (0, 1):
        for w in wnames_bf:
            dparam(f"k{b}_{w}", shapes[w], bf16)
        for w in wnames_f32:
            dparam(f"k{b}_{w}", shapes[w], f32)
    dparam("fw1_m", (128, 64), f32)
    dparam("fw1_l", (1, 64), f32)
    dparam("fb1col", (64, 1), f32)
    dparam("fw2", (64, 1), f32)
    dparam("fb2", (1, 1), f32)
    out_dram = nc.dram_tensor("out", [1, NPC], f32, kind="ExternalOutput")

    with tile.TileContext(nc) as tc, ExitStack() as ctx:
        per = ctx.enter_context(tc.tile_pool(name="per", bufs=1))
        work = ctx.enter_context(tc.tile_pool(name="work", bufs=2))
        ps = ctx.enter_context(tc.tile_pool(name="ps", bufs=4, space="PSUM"))
        dram = ctx.enter_context(tc.tile_pool(name="dram", bufs=1, space="DRAM"))

        idxA = per.tile([128, SP // 16], i16, tag="idxA", name="idxA")
        nc.sync.dma_start(idxA[:], din["idxA"][:])

        im = [per.tile([128, NPC], f32, tag=f"im{i}", name=f"im{i}") for i in range(2)]
        il = [per.tile([1, NPC], f32, tag=f"il{i}", name=f"il{i}") for i in range(2)]
        imb = [per.tile([128, NPC], bf16, tag=f"imb{i}", name=f"imb{i}") for i in range(2)]
        ilb = [per.tile([1, NPC], bf16, tag=f"ilb{i}", name=f"ilb{i}") for i in range(2)]
        nc.sync.dma_start(im[0][:], din["interpT"][0:128, :])
        nc.sync.dma_start(il[0][:], din["interpT"][128:129, :])
        nc.sync.dma_start(imb[0][:], din["interpTb"][0:128, :])
        nc.sync.dma_start(ilb[0][:], din["interpTb"][128:129, :])

        wsb = {}
        for name, t in din.items():
            if name in ("idxA", "a_pad", "interpT", "interpTb"):
                continue
            shp = list(t.shape)
            wsb[name] = per.tile(shp, t.dtype, tag=name, name=name)
            nc.scalar.dma_start(wsb[name][:], t[:])

        pooled = per.tile([128, NPCP], bf16, tag="pooled", name="pooled")
        bT = per.tile([128, NPCP], bf16, tag="bT", name="bT")

        warm_own = dram.tile([32, 32], bf16, name="warm_own")
        warm_full = dram.tile([256, 32], bf16, name="warm_full",
                              addr_space="Shared")
        nc.vector.memset(warm_src := per.tile([32, 32], bf16, tag="warm",
                                              name="warm"), 0.0)
        nc.sync.dma_start(warm_own[:], warm_src[:])
        nc.gpsimd.collective_compute(
            "AllGather", Alu.bypass,
            replica_groups=[list(range(NCORES))],
            ins=[warm_own[:].opt()], outs=[warm_full[:].opt()])

        tab_own = [dram.tile([NPC, H], bf16, name=f"tab_own{i}") for i in range(2)]
        tab_full = [dram.tile([N, H], bf16, name=f"tab_full{i}",
                              addr_space="Shared") for i in range(2)]

        NTT = 124  # 1984 = 16*124

        def build_tables(blk, tt_range):
            """A rows (node-major -> DRAM) and B' (feat-major -> SBUF bT).

            Block 0 runs at startup where DVE is idle but ACT queues behind
            the weight loads, so its PSUM drains go to DVE; block 1 rebuilds
            run while DVE is busy with edge math, so they stay on ACT."""
            curb_m, curb_l = imb[blk % 2], ilb[blk % 2]
            kw = lambda w: wsb[f"k{blk}_{w}"]
            for tt in tt_range:
                sl = slice(tt * NTT, (tt + 1) * NTT)
                psA = ps.tile([NTT, H], f32, tag="psx", name="psx", bufs=3)
                nc.tensor.matmul(psA[:], curb_m[:, sl], kw("w1top_m")[:],
                                 start=True, stop=False)
                nc.tensor.matmul(psA[:], curb_l[:, sl], kw("w1top_l")[:],
                                 start=False, stop=True)
                rA = work.tile([NTT, H], bf16, tag="rowA", name="rowA")
                if blk == 0:
                    nc.vector.tensor_copy(rA[:], psA[:])
                else:
                    nc.scalar.copy(rA[:], psA[:])
                nc.sync.dma_start(tab_own[blk][sl, :], rA[:])

                psB = ps.tile([H, NTT], f32, tag="psh", name="psh", bufs=3)
                nc.tensor.matmul(psB[:], kw("w1mid_m")[:], curb_m[:, sl],
                                 start=True, stop=False)
                nc.tensor.matmul(psB[:], kw("w1mid_l")[:], curb_l[:, sl],
                                 start=False, stop=True)
                if blk == 0:
                    nc.vector.tensor_scalar_add(bT[:, sl], psB[:],
                                                kw("b1col")[:])
                else:
                    nc.scalar.activation(bT[:, sl], psB[:], Act.Identity,
                                         bias=kw("b1col")[:])

        def allgather(blk):
            nc.gpsimd.collective_compute(
                "AllGather", Alu.bypass,
                replica_groups=[list(range(NCORES))],
                ins=[tab_own[blk][:].opt()],
                outs=[tab_full[blk][:].opt()],
            )

        SGT = 496  # 1984 = 4*496

        def blockout_segtile(blk, st):
            kw = lambda w: wsb[f"k{blk}_{w}"]
            cur_m, cur_l = im[blk % 2], il[blk % 2]
            nxt_m, nxt_l = im[(blk + 1) % 2], il[(blk + 1) % 2]
            nxtb_m, nxtb_l = imb[(blk + 1) % 2], ilb[(blk + 1) % 2]
            sl = slice(st * SGT, (st + 1) * SGT)
            po1 = ps.tile([128, SGT], f32, tag="psx", name="psx", bufs=3)
            nc.tensor.matmul(po1[:], kw("wo_m")[:], pooled[:, sl],
                             start=True, stop=True)
            nc.vector.scalar_tensor_tensor(
                nxt_m[:, sl], po1[:], kw("bo_m")[:], cur_m[:, sl],
                op0=Alu.add, op1=Alu.add)
            nc.scalar.copy(nxtb_m[:, sl], nxt_m[:, sl])
            po2 = ps.tile([1, SGT], f32, tag="psh", name="psh", bufs=3)
            nc.tensor.matmul(po2[:], kw("wo_l")[:], pooled[:, sl],
                             start=True, stop=True)
            nc.vector.scalar_tensor_tensor(
                nxt_l[:, sl], po2[:], kw("bo_l")[:], cur_l[:, sl],
                op0=Alu.add, op1=Alu.add)
            nc.scalar.copy(nxtb_l[:, sl], nxt_l[:, sl])

        def bview(node0, nstride, nseg, s):
            """bT[:, node0::nstride][:nseg], broadcast to (128, nseg, s)."""
            v = bT[:, node0:node0 + nseg * nstride]
            v = v.rearrange("p (n k) -> p n k", k=nstride)[:, :, 0:1]
            return v.broadcast_to([128, nseg, s])

        build_tables(0, range(NPC // NTT))
        allgather(0)

        for blk in range(2):
            kw = lambda w: wsb[f"k{blk}_{w}"]
            off = 0
            for ti, t in enumerate(tiles):
                W = t["width"]
                gA = work.tile([128, TILE_SLOTS], bf16, tag="gA", name="gA",
                               bufs=3)
                nc.gpsimd.dma_gather(
                    gA[:, :W].unsqueeze(1), tab_full[blk][:],
                    idxA[:, off // 16:(off + W) // 16],
                    W, W, H, transpose=True, single_packet=False)
                a_sb = work.tile([1, TILE_SLOTS], bf16, tag="a_sb",
                                 name="a_sb", bufs=3)
                nc.sync.dma_start(a_sb[:, :W], din["a_pad"][:, off:off + W])

                # t1 = gA + B'[seg]  (piecewise strided broadcast)
                t1 = work.tile([128, TILE_SLOTS], bf16, tag="t1", name="t1",
                               bufs=2)
                for (s, nseg, node0, nstride, sloff) in t["pieces"]:
                    gv = gA[:, sloff:sloff + nseg * s].rearrange(
                        "p (n k) -> p n k", k=s)
                    tv = t1[:, sloff:sloff + nseg * s].rearrange(
                        "p (n k) -> p n k", k=s)
                    nc.vector.scalar_tensor_tensor(
                        tv, gv, 1.0, bview(node0, nstride, nseg, s),
                        op0=Alu.mult, op1=Alu.add)

                h1 = work.tile([128, TILE_SLOTS], bf16, tag="h1", name="h1",
                               bufs=3)
                h2 = work.tile([128, TILE_SLOTS], bf16, tag="h2", name="h2",
                               bufs=2)
                for s0 in range(0, t["real"], 512):
                    w = min(512, t["real"] - s0)
                    sl = slice(s0, s0 + w)
                    psx = ps.tile([128, 512], f32, tag="psx", name="psx",
                                  bufs=3)
                    nc.tensor.matmul(psx[:, :w], kw("w1r")[:], a_sb[:, sl],
                                     start=True, stop=True)
                    nc.vector.scalar_tensor_tensor(
                        t1[:, sl], psx[:, :w], 1.0, t1[:, sl],
                        op0=Alu.mult, op1=Alu.add)
                    nc.scalar.activation(h1[:, sl], t1[:, sl], Act.Relu)
                    psh = ps.tile([128, 512], f32, tag="psh", name="psh",
                                  bufs=3)
                    nc.tensor.matmul(psh[:, :w], kw("w2")[:], h1[:, sl],
                                     start=True, stop=True)
                    nc.scalar.activation(h2[:, sl], psh[:, :w], Act.Relu,
                                         bias=kw("b2col")[:])

                # ragged segment max: strided reduce per piece
                for (s, nseg, node0, nstride, sloff) in t["pieces"]:
                    src = h2[:, sloff:sloff + nseg * s].rearrange(
                        "p (n k) -> p n k", k=s)
                    dst = pooled[:, node0:node0 + nseg * nstride].rearrange(
                        "p (n k) -> p n k", k=nstride)[:, :, 0]
                    nc.vector.tensor_reduce(dst, src,
                                            axis=mybir.AxisListType.X,
                                            op=Alu.max)
                off += W

            def final_segtile(st):
                # relu(x @ fW1 + fb1) @ fW2 + fb2 for one segment tile
                fin_m, fin_l = im[0], il[0]
                sl = slice(st * SGT, (st + 1) * SGT)
                pz1 = ps.tile([64, SGT], f32, tag="psx", name="psx", bufs=3)
                nc.tensor.matmul(pz1[:], wsb["fw1_m"][:], fin_m[:, sl],
                                 start=True, stop=False)
                nc.tensor.matmul(pz1[:], wsb["fw1_l"][:], fin_l[:, sl],
                                 start=False, stop=True)
                z1 = work.tile([64, SGT], f32, tag="z1", name="z1")
                nc.scalar.activation(z1[:], pz1[:], Act.Relu,
                                     bias=wsb["fb1col"][:])
                pz2 = ps.tile([1, SGT], f32, tag="psh", name="psh", bufs=3)
                nc.tensor.matmul(pz2[:], wsb["fw2"][:], z1[:],
                                 start=True, stop=True)
                osb = work.tile([1, SGT], f32, tag="osb", name="osb")
                nc.scalar.activation(osb[:], pz2[:], Act.Identity,
                                     bias=wsb["fb2"][:])
                nc.sync.dma_start(out_dram[:, sl], osb[:])

            for st in range(NPC // SGT):
                blockout_segtile(blk, st)
                if blk == 0:
                    build_tables(1, range(st * SGT // NTT,
                                          (st + 1) * SGT // NTT))
                else:
                    final_segtile(st)
            if blk == 0:
                allgather(1)

    nc.compile()
    return nc


# ---------------------------------------------------------------------------
# entry point
# ---------------------------------------------------------------------------

def prepare(inputs):
    """Host preprocessing + graph build. Returns (nc, in_maps)."""
    sizes = np.asarray(inputs["neighborhood_sizes"], np.int64)
    nbr = np.asarray(inputs["neighborhoods_indexes"], np.int64)
    a = np.asarray(inputs["add_info"], np.float32)[:, 0]
    interp = np.asarray(inputs["interpolated"], np.float32)

    cores, struct = _preprocess(sizes, nbr, a)
    wmap = _weights_inputs(inputs)

    nc = build_graph(struct)

    in_maps = []
    for c in range(NCORES):
        m = dict(wmap)
        m["idxA"] = cores[c]["idxA"]
        m["a_pad"] = cores[c]["a_pad"]
        m["interpT"] = np.ascontiguousarray(
            interp[c * NPC:(c + 1) * NPC].T)
        m["interpTb"] = m["interpT"].astype(BF16)
        in_maps.append(m)
    return nc, in_maps


def kernel(**inputs):
    from concourse.bass_utils import run_bass_kernel_spmd

    nc, in_maps = prepare(inputs)
    res = run_bass_kernel_spmd(nc, in_maps, core_ids=list(range(NCORES)))
    out = np.concatenate([res.results[c]["out"].reshape(-1)
                          for c in range(NCORES)])
    return out[:, None].astype(np.float32)


if __name__ == "__main__":
    import jax
    cpu = jax.devices("cpu")[0]
    with jax.default_device(cpu):
        import reference as ref
        inp = ref.setup_inputs()
        expected = np.asarray(ref.reference(**inp))
    inp_np = {k: np.asarray(v) for k, v in inp.items()}
    actual = kernel(**inp_np)
    err = np.linalg.norm(actual - expected) / np.linalg.norm(expected)
    print("Relative error:", err)

